# revision 3
# baseline (speedup 1.0000x reference)
"""MiniMaxText01 linear attention layer on 8 trn2 NeuronCores.

Strategy: tensor-parallel over heads (4 heads/core). Per core:
  phase A: qT/kT/gateT/v projections (fp32r matmuls, silu/sigmoid on ACT)
           with all projection weights SBUF-resident, streaming hiddenT
           chunks; results staged to internal DRAM in the layouts the
           attention phase wants ([inner, token] transposed, v natural).
  phase B: blocked lightning attention (BLOCK=256) + gating + out
           projection. kv state [d,128] per head lives in SBUF. The
           RMSNorm rsqrt(var) is a per-token scalar that commutes with
           the out projection, so each core emits
             pout = (gate * attn * norm_w) @ w_out        [4096, 2048]
             ssq  = sum over this core's channels of attn^2   [1, 4096]
           and the host applies out = sum_c(pout) * rsqrt(sum_c(ssq)/4096+eps).
All matmul operands are fp32r (full PE rate at N>=256, ~1e-4 rel err).
"""
import math
import numpy as np
from contextlib import ExitStack

import concourse.bass as bass
import concourse.tile as tile
import concourse.mybir as mybir
from concourse import bacc
from concourse.bass_utils import run_bass_kernel_spmd

FP32 = mybir.dt.float32
FP32R = mybir.dt.float32r
AF = mybir.ActivationFunctionType

SEQ = 4096
HIDDEN = 2048
NUM_HEADS = 32
HEAD_DIM = 128
INNER = NUM_HEADS * HEAD_DIM
BLOCK = 256
EPS = 1e-5
N_CORES = 8
HPC = NUM_HEADS // N_CORES          # 4 heads per core
IN_PC = HPC * HEAD_DIM              # 512 inner channels per core
P = 128

CH_A = 256
NT_A = SEQ // CH_A                  # 16
CH_B = 512
NT_B = SEQ // CH_B                  # 8
HC = HIDDEN // P                    # 16 hidden chunks


def build_nc(repeat: int = 1):
    nc = bacc.Bacc("TRN2", target_bir_lowering=False)

    xt_d = nc.dram_tensor("xt", [HIDDEN, SEQ], FP32, kind="ExternalInput")
    wq_d = nc.dram_tensor("wq", [HIDDEN, IN_PC], FP32, kind="ExternalInput")
    wk_d = nc.dram_tensor("wk", [HIDDEN, IN_PC], FP32, kind="ExternalInput")
    wv_d = nc.dram_tensor("wv", [HIDDEN, IN_PC], FP32, kind="ExternalInput")
    wg_d = nc.dram_tensor("wg", [HIDDEN, IN_PC], FP32, kind="ExternalInput")
    wo_d = nc.dram_tensor("wo", [IN_PC, HIDDEN], FP32, kind="ExternalInput")
    normw_d = nc.dram_tensor("normw", [HPC, P, 1], FP32, kind="ExternalInput")
    qdec_d = nc.dram_tensor("qdec", [HPC, P, BLOCK], FP32, kind="ExternalInput")
    dmask0_d = nc.dram_tensor("dmask0", [HPC, P, BLOCK], FP32, kind="ExternalInput")
    dmask1_d = nc.dram_tensor("dmask1", [HPC, P, BLOCK], FP32, kind="ExternalInput")
    kdec_d = nc.dram_tensor("kdec", [HPC, 2, P, 1], FP32, kind="ExternalInput")
    bd_d = nc.dram_tensor("bd", [P, HPC, 1], FP32, kind="ExternalInput")
    ident_d = nc.dram_tensor("ident", [P, P], FP32, kind="ExternalInput")
    ones_d = nc.dram_tensor("ones", [P, 1], FP32, kind="ExternalInput")
    kv0_d = nc.dram_tensor("kv0", [HPC, P, P], FP32, kind="ExternalInput")

    pout_d = nc.dram_tensor("pout", [SEQ, HIDDEN], FP32, kind="ExternalOutput")
    ssq_d = nc.dram_tensor("ssq", [1, SEQ], FP32, kind="ExternalOutput")

    # internal DRAM intermediates
    qT_d = nc.dram_tensor("qT_i", [IN_PC, SEQ], FP32R)
    kT_d = nc.dram_tensor("kT_i", [IN_PC, SEQ], FP32R)
    v_d = nc.dram_tensor("v_i", [SEQ, IN_PC], FP32R)
    gT_d = nc.dram_tensor("gT_i", [IN_PC, SEQ], FP32)

    with tile.TileContext(nc) as tc, ExitStack() as ctx:
        const = ctx.enter_context(tc.tile_pool(name="const", bufs=1))
        psum = ctx.enter_context(tc.tile_pool(name="psum", bufs=1, space="PSUM"))

        ident_t = const.tile([P, P], FP32R)
        nc.sync.dma_start(ident_t[:], ident_d[:].bitcast(FP32R))
        ones_t = const.tile([P, 1], FP32R)
        nc.sync.dma_start(ones_t[:], ones_d[:].bitcast(FP32R))
        qdec_t = const.tile([P, HPC, BLOCK], FP32)
        nc.sync.dma_start(qdec_t[:], qdec_d[:].rearrange("h p i -> p h i"))
        dmask0_t = const.tile([P, HPC, BLOCK], FP32)
        nc.sync.dma_start(dmask0_t[:], dmask0_d[:].rearrange("h p i -> p h i"))
        dmask1_t = const.tile([P, HPC, BLOCK], FP32)
        nc.sync.dma_start(dmask1_t[:], dmask1_d[:].rearrange("h p i -> p h i"))
        kdec_t = const.tile([P, HPC, 2, 1], FP32)
        nc.sync.dma_start(kdec_t[:], kdec_d[:].rearrange("h s p o -> p h s o"))
        normw_t = const.tile([P, HPC, 1], FP32)
        nc.sync.dma_start(normw_t[:], normw_d[:].rearrange("h p o -> p h o"))
        bd_t = const.tile([P, HPC, 1], FP32)
        nc.sync.dma_start(bd_t[:], bd_d[:])
        kv_t = const.tile([P, HPC, P], FP32R)

        for _rep in range(repeat):
            nc.sync.dma_start(kv_t[:], kv0_d[:].rearrange("h d e -> d h e").bitcast(FP32R))

            # ---------------- phase A: projections ----------------
            with ExitStack() as actx:
                wpool = actx.enter_context(tc.tile_pool(name="wA", bufs=1))
                xpool = actx.enter_context(tc.tile_pool(name="xA", bufs=1))
                stA = actx.enter_context(tc.tile_pool(name="stA", bufs=1))

                wq_t = wpool.tile([P, HC, IN_PC], FP32R)
                nc.sync.dma_start(wq_t[:], wq_d[:].rearrange("(hc p) m -> p hc m", p=P).bitcast(FP32R))
                wk_t = wpool.tile([P, HC, IN_PC], FP32R)
                nc.sync.dma_start(wk_t[:], wk_d[:].rearrange("(hc p) m -> p hc m", p=P).bitcast(FP32R))
                wv_t = wpool.tile([P, HC, IN_PC], FP32R)
                nc.sync.dma_start(wv_t[:], wv_d[:].rearrange("(hc p) m -> p hc m", p=P).bitcast(FP32R))
                wg_t = wpool.tile([P, HC, IN_PC], FP32R)
                nc.sync.dma_start(wg_t[:], wg_d[:].rearrange("(hc p) m -> p hc m", p=P).bitcast(FP32R))

                xt_r = xt_d[:].rearrange("(hc p) n -> p hc n", p=P).bitcast(FP32R)

                for t in range(NT_A):
                    xt_t = xpool.tile([P, HC, CH_A], FP32R, tag="xt", bufs=3)
                    nc.sync.dma_start(xt_t[:], xt_r[:, :, t * CH_A:(t + 1) * CH_A])

                    for w_t, actf, dst, sdt, tg in (
                        (wq_t, AF.Silu, qT_d, FP32R, "sq"),
                        (wk_t, AF.Silu, kT_d, FP32R, "sk"),
                        (wg_t, AF.Sigmoid, gT_d, FP32, "sg"),
                    ):
                        for cc in range(4):
                            ps = psum.tile([P, IN_PC], FP32, tag="psA", bufs=2)
                            for hc in range(HC):
                                nc.tensor.matmul(
                                    ps[:, :CH_A],
                                    w_t[:, hc, cc * P:(cc + 1) * P],
                                    xt_t[:, hc, :],
                                    start=(hc == 0), stop=(hc == HC - 1))
                            sb = stA.tile([P, CH_A], sdt, tag=tg, bufs=3)
                            nc.scalar.activation(sb[:], ps[:, :CH_A], actf)
                            nc.sync.dma_start(
                                dst[cc * P:(cc + 1) * P, t * CH_A:(t + 1) * CH_A], sb[:])

                    for t2 in range(2):
                        ps = psum.tile([P, IN_PC], FP32, tag="psA", bufs=2)
                        for hc in range(HC):
                            nc.tensor.matmul(
                                ps[:],
                                xt_t[:, hc, t2 * P:(t2 + 1) * P],
                                wv_t[:, hc, :],
                                start=(hc == 0), stop=(hc == HC - 1))
                        sb = stA.tile([P, IN_PC], FP32R, tag="sv", bufs=3)
                        nc.scalar.activation(sb[:], ps[:], AF.Silu)
                        nc.sync.dma_start(
                            v_d[t * CH_A + t2 * P: t * CH_A + (t2 + 1) * P, :], sb[:])

            # ---------------- phase B: attention + gating + out ----------------
            with ExitStack() as bctx:
                wBpool = bctx.enter_context(tc.tile_pool(name="wB", bufs=1))
                sB = bctx.enter_context(tc.tile_pool(name="sB", bufs=1))
                wk_b = bctx.enter_context(tc.tile_pool(name="wkB", bufs=1))

                wo_t = wBpool.tile([P, HPC, HIDDEN], FP32R)
                nc.sync.dma_start(wo_t[:], wo_d[:].rearrange("(h p) n -> p h n", p=P).bitcast(FP32R))

                qT_r = qT_d[:].rearrange("(h p) n -> p h n", p=P)
                kT_r = kT_d[:].rearrange("(h p) n -> p h n", p=P)
                gT_r = gT_d[:].rearrange("(h p) n -> p h n", p=P)

                for t in range(NT_B):
                    tsl = slice(t * CH_B, (t + 1) * CH_B)
                    q_t = sB.tile([P, HPC, CH_B], FP32R, tag="q", bufs=2)
                    nc.sync.dma_start(q_t[:], qT_r[:, :, tsl])
                    k_t = sB.tile([P, HPC, CH_B], FP32R, tag="k", bufs=2)
                    nc.sync.dma_start(k_t[:], kT_r[:, :, tsl])
                    g_t = sB.tile([P, HPC, CH_B], FP32, tag="g", bufs=2)
                    nc.sync.dma_start(g_t[:], gT_r[:, :, tsl])
                    v_t = sB.tile([P, 4, IN_PC], FP32R, tag="v", bufs=2)
                    nc.sync.dma_start(
                        v_t[:], v_d[tsl, :].rearrange("(s p) c -> p s c", p=P))

                    attn_t = wk_b.tile([P, HPC, CH_B], FP32, tag="attn", bufs=2)

                    for b in range(CH_B // BLOCK):
                        t0 = b * BLOCK
                        for h in range(HPC):
                            hsl = slice(h * P, (h + 1) * P)
                            # scores (transposed): sT[j, i] = k_j . q_i
                            ps0 = psum.tile([P, BLOCK], FP32, tag="ps_s", bufs=2)
                            nc.tensor.matmul(ps0[:], k_t[:, h, t0:t0 + P],
                                             q_t[:, h, t0:t0 + BLOCK],
                                             start=True, stop=True)
                            ps1 = psum.tile([P, BLOCK], FP32, tag="ps_s", bufs=2)
                            nc.tensor.matmul(ps1[:], k_t[:, h, t0 + P:t0 + BLOCK],
                                             q_t[:, h, t0:t0 + BLOCK],
                                             start=True, stop=True)
                            s0 = wk_b.tile([P, BLOCK], FP32R, tag="s0", bufs=2)
                            nc.vector.tensor_mul(s0[:], ps0[:], dmask0_t[:, h, :])
                            s1 = wk_b.tile([P, BLOCK], FP32R, tag="s1", bufs=2)
                            nc.vector.tensor_mul(s1[:], ps1[:], dmask1_t[:, h, :])
                            qd = wk_b.tile([P, BLOCK], FP32R, tag="qd", bufs=2)
                            nc.vector.tensor_mul(qd[:], q_t[:, h, t0:t0 + BLOCK].bitcast(FP32),
                                                 qdec_t[:, h, :])
                            # k natural (transposed back) with k-decay folded in
                            kn = []
                            for sub in range(2):
                                pst = psum.tile([P, P], FP32, tag="ps_tr", bufs=1)
                                nc.tensor.transpose(
                                    pst[:].bitcast(FP32R),
                                    k_t[:, h, t0 + sub * P:t0 + (sub + 1) * P],
                                    ident_t[:])
                                knt = wk_b.tile([P, P], FP32R, tag=f"kn{sub}", bufs=2)
                                nc.scalar.activation(knt[:], pst[:], AF.Copy,
                                                     scale=kdec_t[:, h, sub, :])
                                kn.append(knt)
                            # attention output (transposed): inter + intra
                            pso = psum.tile([P, BLOCK], FP32, tag="ps_o", bufs=1)
                            nc.tensor.matmul(pso[:], kv_t[:, h, :], qd[:],
                                             start=True, stop=False)
                            nc.tensor.matmul(pso[:], v_t[:, 2 * b, hsl], s0[:],
                                             start=False, stop=False)
                            nc.tensor.matmul(pso[:], v_t[:, 2 * b + 1, hsl], s1[:],
                                             start=False, stop=True)
                            nc.scalar.copy(attn_t[:, h, t0:t0 + BLOCK], pso[:])
                            # kv update: kv = bd*kv + (k*kdec)^T v
                            psk = psum.tile([P, P], FP32, tag="ps_kv", bufs=1)
                            nc.tensor.matmul(psk[:], kn[0][:], v_t[:, 2 * b, hsl],
                                             start=True, stop=False)
                            nc.tensor.matmul(psk[:], kn[1][:], v_t[:, 2 * b + 1, hsl],
                                             start=False, stop=True)
                            nc.vector.tensor_scalar_mul(
                                kv_t[:, h, :], kv_t[:, h, :].bitcast(FP32), bd_t[:, h, :])
                            nc.vector.tensor_add(
                                kv_t[:, h, :], kv_t[:, h, :].bitcast(FP32), psk[:])

                    # gating + norm weight + squares
                    gA_t = wk_b.tile([P, HPC, CH_B], FP32R, tag="gA", bufs=2)
                    sq_t = wk_b.tile([P, HPC, CH_B], FP32R, tag="sq", bufs=2)
                    for h in range(HPC):
                        nc.vector.tensor_mul(gA_t[:, h, :], attn_t[:, h, :], g_t[:, h, :])
                        nc.vector.tensor_scalar_mul(
                            gA_t[:, h, :], gA_t[:, h, :].bitcast(FP32), normw_t[:, h, :])
                        nc.vector.tensor_mul(sq_t[:, h, :], attn_t[:, h, :], attn_t[:, h, :])

                    # ssq = ones^T @ sq  (sum over this core's inner channels)
                    pss = psum.tile([1, CH_B], FP32, tag="psA", bufs=2)
                    for h in range(HPC):
                        nc.tensor.matmul(pss[:], ones_t[:], sq_t[:, h, :],
                                         start=(h == 0), stop=(h == HPC - 1))
                    ssb = wk_b.tile([1, CH_B], FP32, tag="ssb", bufs=2)
                    nc.scalar.copy(ssb[:], pss[:])
                    nc.sync.dma_start(ssq_d[:, tsl], ssb[:])

                    # out projection
                    for m in range(4):
                        ob = wk_b.tile([P, HIDDEN], FP32, tag="ob", bufs=2)
                        for nt in range(4):
                            pso2 = psum.tile([P, 512], FP32, tag="psA", bufs=2)
                            for h in range(HPC):
                                nc.tensor.matmul(
                                    pso2[:],
                                    gA_t[:, h, m * P:(m + 1) * P],
                                    wo_t[:, h, nt * 512:(nt + 1) * 512],
                                    start=(h == 0), stop=(h == HPC - 1))
                            nc.scalar.copy(ob[:, nt * 512:(nt + 1) * 512], pso2[:])
                        nc.sync.dma_start(
                            pout_d[t * CH_B + m * P: t * CH_B + (m + 1) * P, :], ob[:])

    nc.compile()
    return nc


_NC_CACHE = {}


def _get_nc(repeat=1):
    if repeat not in _NC_CACHE:
        _NC_CACHE[repeat] = build_nc(repeat)
    return _NC_CACHE[repeat]


def make_in_maps(inputs):
    hs = np.ascontiguousarray(np.asarray(inputs["hidden_states"], dtype=np.float32))
    w_qkv = np.asarray(inputs["w_qkv"], dtype=np.float32)
    w_gate = np.asarray(inputs["w_gate"], dtype=np.float32)
    w_out = np.asarray(inputs["w_out"], dtype=np.float32)
    norm_weight = np.asarray(inputs["norm_weight"], dtype=np.float32)
    slope_rate = np.asarray(inputs["slope_rate"], dtype=np.float32).reshape(NUM_HEADS)
    kv_cache = np.asarray(inputs["kv_cache"], dtype=np.float32)

    xt = np.ascontiguousarray(hs.T)                      # [HIDDEN, SEQ]
    wq3 = w_qkv.reshape(HIDDEN, NUM_HEADS, 3 * HEAD_DIM)
    ident = np.eye(P, dtype=np.float32)
    ones = np.ones((P, 1), dtype=np.float32)
    idx = np.arange(BLOCK, dtype=np.float64)

    in_maps = []
    for c in range(N_CORES):
        heads = range(c * HPC, (c + 1) * HPC)
        s = slope_rate[c * HPC:(c + 1) * HPC].astype(np.float64)  # [HPC]
        wq = np.ascontiguousarray(
            wq3[:, c * HPC:(c + 1) * HPC, 0:HEAD_DIM].reshape(HIDDEN, IN_PC))
        wk = np.ascontiguousarray(
            wq3[:, c * HPC:(c + 1) * HPC, HEAD_DIM:2 * HEAD_DIM].reshape(HIDDEN, IN_PC))
        wv = np.ascontiguousarray(
            wq3[:, c * HPC:(c + 1) * HPC, 2 * HEAD_DIM:3 * HEAD_DIM].reshape(HIDDEN, IN_PC))
        wg = np.ascontiguousarray(w_gate[:, c * IN_PC:(c + 1) * IN_PC])
        wo = np.ascontiguousarray(w_out[c * IN_PC:(c + 1) * IN_PC, :])
        normw = np.ascontiguousarray(
            norm_weight[c * IN_PC:(c + 1) * IN_PC].reshape(HPC, P, 1))

        diff = idx[:128, None] - idx[None, :]            # j - i restricted later
        # dmask0[h, j, i] = exp(-s (i - j)) for i >= j (j in 0..127, i in 0..255)
        jj = idx[:128][:, None]                          # [128,1]
        ii = idx[None, :]                                # [1,256]
        d0 = np.exp(-s[:, None, None] * (ii - jj)) * (ii >= jj)
        dmask0 = d0.astype(np.float32)                   # [HPC,128,256]
        # dmask1[h, j', i] for abs j = j'+128: zero for i<128, else dmask0[j', i-128]
        dmask1 = np.zeros((HPC, P, BLOCK), dtype=np.float32)
        dmask1[:, :, P:] = dmask0[:, :, :P]
        qdec = np.broadcast_to(
            np.exp(-s[:, None] * (idx[None, :] + 1.0))[:, None, :],
            (HPC, P, BLOCK)).astype(np.float32)
        kdec = np.exp(-s[:, None] * (BLOCK - 1.0 - idx[None, :]))  # [HPC, 256]
        kdec = kdec.reshape(HPC, 2, P, 1).astype(np.float32)
        bd = np.broadcast_to(
            np.exp(-s * BLOCK).astype(np.float32)[None, :, None], (P, HPC, 1))
        kv0 = np.ascontiguousarray(kv_cache[c * HPC:(c + 1) * HPC])

        in_maps.append({
            "xt": xt, "wq": wq, "wk": wk, "wv": wv, "wg": wg, "wo": wo,
            "normw": normw, "qdec": np.ascontiguousarray(qdec),
            "dmask0": dmask0, "dmask1": dmask1, "kdec": kdec,
            "bd": np.ascontiguousarray(bd), "ident": ident, "ones": ones,
            "kv0": kv0,
        })
    return in_maps


def combine_outputs(results):
    pout = np.zeros((SEQ, HIDDEN), dtype=np.float64)
    ssq = np.zeros((SEQ,), dtype=np.float64)
    for r in results:
        pout += r["pout"].astype(np.float64)
        ssq += r["ssq"].reshape(SEQ).astype(np.float64)
    var = ssq / INNER
    scale = 1.0 / np.sqrt(var + EPS)
    return (pout * scale[:, None]).astype(np.float32)


def kernel(**inputs):
    nc = _get_nc(1)
    in_maps = make_in_maps(inputs)
    res = run_bass_kernel_spmd(nc, in_maps, core_ids=list(range(N_CORES)))
    return combine_outputs(res.results)


# revision 4
# speedup vs baseline: 6.2008x; 6.2008x over previous
"""MiniMaxText01 linear attention layer on 8 trn2 NeuronCores.

Strategy: tensor-parallel over heads (4 heads/core). Per core:
  phase A: qT/kT/gateT/v projections (fp32r matmuls, silu/sigmoid on ACT)
           with all projection weights SBUF-resident, streaming hiddenT
           chunks; results staged to internal DRAM in the layouts the
           attention phase wants ([inner, token] transposed, v natural).
  phase B: blocked lightning attention (BLOCK=256) + gating + out
           projection. kv state [d,128] per head lives in SBUF. The
           RMSNorm rsqrt(var) is a per-token scalar that commutes with
           the out projection, so each core emits
             pout = (gate * attn * norm_w) @ w_out        [4096, 2048]
             ssq  = sum over this core's channels of attn^2   [1, 4096]
           and the host applies out = sum_c(pout) * rsqrt(sum_c(ssq)/4096+eps).
All matmul operands are fp32r (full PE rate at N>=256, ~1e-4 rel err).
"""
import math
import numpy as np
from contextlib import ExitStack

import concourse.bass as bass
import concourse.tile as tile
import concourse.mybir as mybir
from concourse import bacc
from concourse.bass_utils import run_bass_kernel_spmd

FP32 = mybir.dt.float32
FP32R = mybir.dt.float32r
AF = mybir.ActivationFunctionType

SEQ = 4096
HIDDEN = 2048
NUM_HEADS = 32
HEAD_DIM = 128
INNER = NUM_HEADS * HEAD_DIM
BLOCK = 256
EPS = 1e-5
N_CORES = 8
HPC = NUM_HEADS // N_CORES          # 4 heads per core
IN_PC = HPC * HEAD_DIM              # 512 inner channels per core
P = 128

CH_A = 256
NT_A = SEQ // CH_A                  # 16
CH_B = 512
NT_B = SEQ // CH_B                  # 8
HC = HIDDEN // P                    # 16 hidden chunks


def build_nc(repeat: int = 1, phases: str = "AB"):
    nc = bacc.Bacc("TRN2", target_bir_lowering=False)

    xt_d = nc.dram_tensor("xt", [HIDDEN, SEQ], FP32, kind="ExternalInput")
    wq_d = nc.dram_tensor("wq", [HIDDEN, IN_PC], FP32, kind="ExternalInput")
    wk_d = nc.dram_tensor("wk", [HIDDEN, IN_PC], FP32, kind="ExternalInput")
    wv_d = nc.dram_tensor("wv", [HIDDEN, IN_PC], FP32, kind="ExternalInput")
    wg_d = nc.dram_tensor("wg", [HIDDEN, IN_PC], FP32, kind="ExternalInput")
    wo_d = nc.dram_tensor("wo", [IN_PC, HIDDEN], FP32, kind="ExternalInput")
    normw_d = nc.dram_tensor("normw", [HPC, P, 1], FP32, kind="ExternalInput")
    qdec_d = nc.dram_tensor("qdec", [HPC, P, BLOCK], FP32, kind="ExternalInput")
    dmask0_d = nc.dram_tensor("dmask0", [HPC, P, BLOCK], FP32, kind="ExternalInput")
    dmask1_d = nc.dram_tensor("dmask1", [HPC, P, BLOCK], FP32, kind="ExternalInput")
    kdec_d = nc.dram_tensor("kdec", [HPC, 2, P, 1], FP32, kind="ExternalInput")
    bd_d = nc.dram_tensor("bd", [P, HPC, 1], FP32, kind="ExternalInput")
    ident_d = nc.dram_tensor("ident", [P, P], FP32, kind="ExternalInput")
    ones_d = nc.dram_tensor("ones", [P, 1], FP32, kind="ExternalInput")
    kv0_d = nc.dram_tensor("kv0", [HPC, P, P], FP32, kind="ExternalInput")

    pout_d = nc.dram_tensor("pout", [SEQ, HIDDEN], FP32, kind="ExternalOutput")
    ssq_d = nc.dram_tensor("ssq", [1, SEQ], FP32, kind="ExternalOutput")

    # internal DRAM intermediates
    qT_d = nc.dram_tensor("qT_i", [IN_PC, SEQ], FP32R)
    kT_d = nc.dram_tensor("kT_i", [IN_PC, SEQ], FP32R)
    v_d = nc.dram_tensor("v_i", [SEQ, IN_PC], FP32R)
    gT_d = nc.dram_tensor("gT_i", [IN_PC, SEQ], FP32)

    with tile.TileContext(nc) as tc, ExitStack() as ctx:
        const = ctx.enter_context(tc.tile_pool(name="const", bufs=1))
        psum = ctx.enter_context(tc.tile_pool(name="psum", bufs=1, space="PSUM"))

        ident_t = const.tile([P, P], FP32R)
        nc.sync.dma_start(ident_t[:], ident_d[:].bitcast(FP32R))
        ones_t = const.tile([P, 1], FP32R)
        nc.sync.dma_start(ones_t[:], ones_d[:].bitcast(FP32R))
        qdec_t = const.tile([P, HPC, BLOCK], FP32)
        nc.sync.dma_start(qdec_t[:], qdec_d[:].rearrange("h p i -> p h i"))
        dmask0_t = const.tile([P, HPC, BLOCK], FP32)
        nc.sync.dma_start(dmask0_t[:], dmask0_d[:].rearrange("h p i -> p h i"))
        dmask1_t = const.tile([P, HPC, BLOCK], FP32)
        nc.sync.dma_start(dmask1_t[:], dmask1_d[:].rearrange("h p i -> p h i"))
        kdec_t = const.tile([P, HPC, 2, 1], FP32)
        nc.sync.dma_start(kdec_t[:], kdec_d[:].rearrange("h s p o -> p h s o"))
        normw_t = const.tile([P, HPC, 1], FP32)
        nc.sync.dma_start(normw_t[:], normw_d[:].rearrange("h p o -> p h o"))
        bd_t = const.tile([P, HPC, 1], FP32)
        nc.sync.dma_start(bd_t[:], bd_d[:])
        kv_t = const.tile([P, HPC, P], FP32R)

        for _rep in range(repeat):
            nc.sync.dma_start(kv_t[:], kv0_d[:].rearrange("h d e -> d h e").bitcast(FP32R))

            # ---------------- phase A: projections ----------------
            if "A" not in phases:
                pass
            else:
              with ExitStack() as actx:
                wpool = actx.enter_context(tc.tile_pool(name="wA", bufs=1))
                xpool = actx.enter_context(tc.tile_pool(name="xA", bufs=1))
                stA = actx.enter_context(tc.tile_pool(name="stA", bufs=1))

                wq_t = wpool.tile([P, HC, IN_PC], FP32R)
                nc.sync.dma_start(wq_t[:], wq_d[:].rearrange("(hc p) m -> p hc m", p=P).bitcast(FP32R))
                wk_t = wpool.tile([P, HC, IN_PC], FP32R)
                nc.sync.dma_start(wk_t[:], wk_d[:].rearrange("(hc p) m -> p hc m", p=P).bitcast(FP32R))
                wv_t = wpool.tile([P, HC, IN_PC], FP32R)
                nc.sync.dma_start(wv_t[:], wv_d[:].rearrange("(hc p) m -> p hc m", p=P).bitcast(FP32R))
                wg_t = wpool.tile([P, HC, IN_PC], FP32R)
                nc.sync.dma_start(wg_t[:], wg_d[:].rearrange("(hc p) m -> p hc m", p=P).bitcast(FP32R))

                xt_r = xt_d[:].rearrange("(hc p) n -> p hc n", p=P).bitcast(FP32R)

                for t in range(NT_A):
                    xt_t = xpool.tile([P, HC, CH_A], FP32R, tag="xt", bufs=3)
                    nc.sync.dma_start(xt_t[:], xt_r[:, :, t * CH_A:(t + 1) * CH_A])

                    for w_t, actf, dst, sdt, tg in (
                        (wq_t, AF.Silu, qT_d, FP32R, "sq"),
                        (wk_t, AF.Silu, kT_d, FP32R, "sk"),
                        (wg_t, AF.Sigmoid, gT_d, FP32, "sg"),
                    ):
                        for cc in range(4):
                            ps = psum.tile([P, IN_PC], FP32, tag="psA", bufs=2)
                            for hc in range(HC):
                                nc.tensor.matmul(
                                    ps[:, :CH_A],
                                    w_t[:, hc, cc * P:(cc + 1) * P],
                                    xt_t[:, hc, :],
                                    start=(hc == 0), stop=(hc == HC - 1))
                            sb = stA.tile([P, CH_A], sdt, tag=tg, bufs=3)
                            nc.scalar.activation(sb[:], ps[:, :CH_A], actf)
                            nc.sync.dma_start(
                                dst[cc * P:(cc + 1) * P, t * CH_A:(t + 1) * CH_A], sb[:])

                    for t2 in range(2):
                        ps = psum.tile([P, IN_PC], FP32, tag="psA", bufs=2)
                        for hc in range(HC):
                            nc.tensor.matmul(
                                ps[:],
                                xt_t[:, hc, t2 * P:(t2 + 1) * P],
                                wv_t[:, hc, :],
                                start=(hc == 0), stop=(hc == HC - 1))
                        sb = stA.tile([P, IN_PC], FP32R, tag="sv", bufs=3)
                        nc.scalar.activation(sb[:], ps[:], AF.Silu)
                        nc.sync.dma_start(
                            v_d[t * CH_A + t2 * P: t * CH_A + (t2 + 1) * P, :], sb[:])

            # ---------------- phase B: attention + gating + out ----------------
            if "B" not in phases:
                pass
            else:
              with ExitStack() as bctx:
                wBpool = bctx.enter_context(tc.tile_pool(name="wB", bufs=1))
                sB = bctx.enter_context(tc.tile_pool(name="sB", bufs=1))
                wk_b = bctx.enter_context(tc.tile_pool(name="wkB", bufs=1))

                wo_t = wBpool.tile([P, HPC, HIDDEN], FP32R)
                nc.sync.dma_start(wo_t[:], wo_d[:].rearrange("(h p) n -> p h n", p=P).bitcast(FP32R))

                qT_r = qT_d[:].rearrange("(h p) n -> p h n", p=P)
                kT_r = kT_d[:].rearrange("(h p) n -> p h n", p=P)
                gT_r = gT_d[:].rearrange("(h p) n -> p h n", p=P)

                for t in range(NT_B):
                    tsl = slice(t * CH_B, (t + 1) * CH_B)
                    q_t = sB.tile([P, HPC, CH_B], FP32R, tag="q", bufs=2)
                    nc.sync.dma_start(q_t[:], qT_r[:, :, tsl])
                    k_t = sB.tile([P, HPC, CH_B], FP32R, tag="k", bufs=2)
                    nc.sync.dma_start(k_t[:], kT_r[:, :, tsl])
                    g_t = sB.tile([P, HPC, CH_B], FP32, tag="g", bufs=2)
                    nc.sync.dma_start(g_t[:], gT_r[:, :, tsl])
                    v_t = sB.tile([P, 4, IN_PC], FP32R, tag="v", bufs=2)
                    nc.sync.dma_start(
                        v_t[:], v_d[tsl, :].rearrange("(s p) c -> p s c", p=P))

                    attn_t = wk_b.tile([P, HPC, CH_B], FP32, tag="attn", bufs=2)

                    for b in range(CH_B // BLOCK):
                        t0 = b * BLOCK
                        for h in range(HPC):
                            hsl = slice(h * P, (h + 1) * P)
                            # scores (transposed): sT[j, i] = k_j . q_i
                            ps0 = psum.tile([P, BLOCK], FP32, tag="ps_s", bufs=2)
                            nc.tensor.matmul(ps0[:], k_t[:, h, t0:t0 + P],
                                             q_t[:, h, t0:t0 + BLOCK],
                                             start=True, stop=True)
                            ps1 = psum.tile([P, BLOCK], FP32, tag="ps_s", bufs=2)
                            nc.tensor.matmul(ps1[:], k_t[:, h, t0 + P:t0 + BLOCK],
                                             q_t[:, h, t0:t0 + BLOCK],
                                             start=True, stop=True)
                            s0 = wk_b.tile([P, BLOCK], FP32R, tag="s0", bufs=2)
                            nc.vector.tensor_mul(s0[:], ps0[:], dmask0_t[:, h, :])
                            s1 = wk_b.tile([P, BLOCK], FP32R, tag="s1", bufs=2)
                            nc.vector.tensor_mul(s1[:], ps1[:], dmask1_t[:, h, :])
                            qd = wk_b.tile([P, BLOCK], FP32R, tag="qd", bufs=2)
                            nc.vector.tensor_mul(qd[:], q_t[:, h, t0:t0 + BLOCK].bitcast(FP32),
                                                 qdec_t[:, h, :])
                            # k natural (transposed back) with k-decay folded in
                            kn = []
                            for sub in range(2):
                                pst = psum.tile([P, P], FP32, tag="ps_tr", bufs=1)
                                nc.tensor.transpose(
                                    pst[:].bitcast(FP32R),
                                    k_t[:, h, t0 + sub * P:t0 + (sub + 1) * P],
                                    ident_t[:])
                                knt = wk_b.tile([P, P], FP32R, tag=f"kn{sub}", bufs=2)
                                nc.scalar.activation(knt[:], pst[:], AF.Copy,
                                                     scale=kdec_t[:, h, sub, :])
                                kn.append(knt)
                            # attention output (transposed): inter + intra
                            pso = psum.tile([P, BLOCK], FP32, tag="ps_o", bufs=1)
                            nc.tensor.matmul(pso[:], kv_t[:, h, :], qd[:],
                                             start=True, stop=False)
                            nc.tensor.matmul(pso[:], v_t[:, 2 * b, hsl], s0[:],
                                             start=False, stop=False)
                            nc.tensor.matmul(pso[:], v_t[:, 2 * b + 1, hsl], s1[:],
                                             start=False, stop=True)
                            nc.scalar.copy(attn_t[:, h, t0:t0 + BLOCK], pso[:])
                            # kv update: kv = bd*kv + (k*kdec)^T v
                            psk = psum.tile([P, P], FP32, tag="ps_kv", bufs=1)
                            nc.tensor.matmul(psk[:], kn[0][:], v_t[:, 2 * b, hsl],
                                             start=True, stop=False)
                            nc.tensor.matmul(psk[:], kn[1][:], v_t[:, 2 * b + 1, hsl],
                                             start=False, stop=True)
                            nc.vector.tensor_scalar_mul(
                                kv_t[:, h, :], kv_t[:, h, :].bitcast(FP32), bd_t[:, h, :])
                            nc.vector.tensor_add(
                                kv_t[:, h, :], kv_t[:, h, :].bitcast(FP32), psk[:])

                    # gating + norm weight + squares
                    gA_t = wk_b.tile([P, HPC, CH_B], FP32R, tag="gA", bufs=2)
                    sq_t = wk_b.tile([P, HPC, CH_B], FP32R, tag="sq", bufs=2)
                    for h in range(HPC):
                        nc.vector.tensor_mul(gA_t[:, h, :], attn_t[:, h, :], g_t[:, h, :])
                        nc.vector.tensor_scalar_mul(
                            gA_t[:, h, :], gA_t[:, h, :].bitcast(FP32), normw_t[:, h, :])
                        nc.vector.tensor_mul(sq_t[:, h, :], attn_t[:, h, :], attn_t[:, h, :])

                    # ssq = ones^T @ sq  (sum over this core's inner channels)
                    pss = psum.tile([1, CH_B], FP32, tag="psA", bufs=2)
                    for h in range(HPC):
                        nc.tensor.matmul(pss[:], ones_t[:], sq_t[:, h, :],
                                         start=(h == 0), stop=(h == HPC - 1))
                    ssb = wk_b.tile([1, CH_B], FP32, tag="ssb", bufs=2)
                    nc.scalar.copy(ssb[:], pss[:])
                    nc.sync.dma_start(ssq_d[:, tsl], ssb[:])

                    # out projection
                    for m in range(4):
                        ob = wk_b.tile([P, HIDDEN], FP32, tag="ob", bufs=2)
                        for nt in range(4):
                            pso2 = psum.tile([P, 512], FP32, tag="psA", bufs=2)
                            for h in range(HPC):
                                nc.tensor.matmul(
                                    pso2[:],
                                    gA_t[:, h, m * P:(m + 1) * P],
                                    wo_t[:, h, nt * 512:(nt + 1) * 512],
                                    start=(h == 0), stop=(h == HPC - 1))
                            nc.scalar.copy(ob[:, nt * 512:(nt + 1) * 512], pso2[:])
                        nc.sync.dma_start(
                            pout_d[t * CH_B + m * P: t * CH_B + (m + 1) * P, :], ob[:])

    nc.compile()
    return nc


_NC_CACHE = {}


def _get_nc(repeat=1, phases="AB"):
    key = (repeat, phases)
    if key not in _NC_CACHE:
        _NC_CACHE[key] = build_nc(repeat, phases)
    return _NC_CACHE[key]


def make_in_maps(inputs):
    hs = np.ascontiguousarray(np.asarray(inputs["hidden_states"], dtype=np.float32))
    w_qkv = np.asarray(inputs["w_qkv"], dtype=np.float32)
    w_gate = np.asarray(inputs["w_gate"], dtype=np.float32)
    w_out = np.asarray(inputs["w_out"], dtype=np.float32)
    norm_weight = np.asarray(inputs["norm_weight"], dtype=np.float32)
    slope_rate = np.asarray(inputs["slope_rate"], dtype=np.float32).reshape(NUM_HEADS)
    kv_cache = np.asarray(inputs["kv_cache"], dtype=np.float32)

    xt = np.ascontiguousarray(hs.T)                      # [HIDDEN, SEQ]
    wq3 = w_qkv.reshape(HIDDEN, NUM_HEADS, 3 * HEAD_DIM)
    ident = np.eye(P, dtype=np.float32)
    ones = np.ones((P, 1), dtype=np.float32)
    idx = np.arange(BLOCK, dtype=np.float64)

    in_maps = []
    for c in range(N_CORES):
        heads = range(c * HPC, (c + 1) * HPC)
        s = slope_rate[c * HPC:(c + 1) * HPC].astype(np.float64)  # [HPC]
        wq = np.ascontiguousarray(
            wq3[:, c * HPC:(c + 1) * HPC, 0:HEAD_DIM].reshape(HIDDEN, IN_PC))
        wk = np.ascontiguousarray(
            wq3[:, c * HPC:(c + 1) * HPC, HEAD_DIM:2 * HEAD_DIM].reshape(HIDDEN, IN_PC))
        wv = np.ascontiguousarray(
            wq3[:, c * HPC:(c + 1) * HPC, 2 * HEAD_DIM:3 * HEAD_DIM].reshape(HIDDEN, IN_PC))
        wg = np.ascontiguousarray(w_gate[:, c * IN_PC:(c + 1) * IN_PC])
        wo = np.ascontiguousarray(w_out[c * IN_PC:(c + 1) * IN_PC, :])
        normw = np.ascontiguousarray(
            norm_weight[c * IN_PC:(c + 1) * IN_PC].reshape(HPC, P, 1))

        diff = idx[:128, None] - idx[None, :]            # j - i restricted later
        # dmask0[h, j, i] = exp(-s (i - j)) for i >= j (j in 0..127, i in 0..255)
        jj = idx[:128][:, None]                          # [128,1]
        ii = idx[None, :]                                # [1,256]
        d0 = np.exp(-s[:, None, None] * (ii - jj)) * (ii >= jj)
        dmask0 = d0.astype(np.float32)                   # [HPC,128,256]
        # dmask1[h, j', i] for abs j = j'+128: zero for i<128, else dmask0[j', i-128]
        dmask1 = np.zeros((HPC, P, BLOCK), dtype=np.float32)
        dmask1[:, :, P:] = dmask0[:, :, :P]
        qdec = np.broadcast_to(
            np.exp(-s[:, None] * (idx[None, :] + 1.0))[:, None, :],
            (HPC, P, BLOCK)).astype(np.float32)
        kdec = np.exp(-s[:, None] * (BLOCK - 1.0 - idx[None, :]))  # [HPC, 256]
        kdec = kdec.reshape(HPC, 2, P, 1).astype(np.float32)
        bd = np.broadcast_to(
            np.exp(-s * BLOCK).astype(np.float32)[None, :, None], (P, HPC, 1))
        kv0 = np.ascontiguousarray(kv_cache[c * HPC:(c + 1) * HPC])

        in_maps.append({
            "xt": xt, "wq": wq, "wk": wk, "wv": wv, "wg": wg, "wo": wo,
            "normw": normw, "qdec": np.ascontiguousarray(qdec),
            "dmask0": dmask0, "dmask1": dmask1, "kdec": kdec,
            "bd": np.ascontiguousarray(bd), "ident": ident, "ones": ones,
            "kv0": kv0,
        })
    return in_maps


def combine_outputs(results):
    pout = np.zeros((SEQ, HIDDEN), dtype=np.float64)
    ssq = np.zeros((SEQ,), dtype=np.float64)
    for r in results:
        pout += r["pout"].astype(np.float64)
        ssq += r["ssq"].reshape(SEQ).astype(np.float64)
    var = ssq / INNER
    scale = 1.0 / np.sqrt(var + EPS)
    return (pout * scale[:, None]).astype(np.float32)


def kernel(**inputs):
    nc = _get_nc(1)
    in_maps = make_in_maps(inputs)
    res = run_bass_kernel_spmd(nc, in_maps, core_ids=list(range(N_CORES)))
    return combine_outputs(res.results)


# revision 6
# speedup vs baseline: 1430.3052x; 230.6644x over previous
"""MiniMaxText01 linear attention layer on 8 trn2 NeuronCores.

Strategy: tensor-parallel over heads (4 heads/core). Per core:
  phase A: qT/kT/gateT/v projections (fp32r matmuls, silu/sigmoid on ACT)
           with all projection weights SBUF-resident, streaming hiddenT
           chunks; results staged to internal DRAM in the layouts the
           attention phase wants ([inner, token] transposed, v natural).
  phase B: blocked lightning attention (BLOCK=256) + gating + out
           projection. kv state [d,128] per head lives in SBUF. The
           RMSNorm rsqrt(var) is a per-token scalar that commutes with
           the out projection, so each core emits
             pout = (gate * attn * norm_w) @ w_out        [4096, 2048]
             ssq  = sum over this core's channels of attn^2   [1, 4096]
           and the host applies out = sum_c(pout) * rsqrt(sum_c(ssq)/4096+eps).
All matmul operands are fp32r (full PE rate at N>=256, ~1e-4 rel err).
"""
import math
import numpy as np
from contextlib import ExitStack

import concourse.bass as bass
import concourse.tile as tile
import concourse.mybir as mybir
from concourse import bacc
from concourse.bass_utils import run_bass_kernel_spmd

FP32 = mybir.dt.float32
FP32R = mybir.dt.float32r
AF = mybir.ActivationFunctionType

SEQ = 4096
HIDDEN = 2048
NUM_HEADS = 32
HEAD_DIM = 128
INNER = NUM_HEADS * HEAD_DIM
BLOCK = 256
EPS = 1e-5
N_CORES = 8
HPC = NUM_HEADS // N_CORES          # 4 heads per core
IN_PC = HPC * HEAD_DIM              # 512 inner channels per core
P = 128

CH_A = 512
NT_A = SEQ // CH_A                  # 8
CH_B = 512
NT_B = SEQ // CH_B                  # 8
HC = HIDDEN // P                    # 16 hidden chunks


def build_nc(repeat: int = 1, phases: str = "AB"):
    nc = bacc.Bacc("TRN2", target_bir_lowering=False)

    xt_d = nc.dram_tensor("xt", [HIDDEN, SEQ], FP32, kind="ExternalInput")
    wq_d = nc.dram_tensor("wq", [HIDDEN, IN_PC], FP32, kind="ExternalInput")
    wk_d = nc.dram_tensor("wk", [HIDDEN, IN_PC], FP32, kind="ExternalInput")
    wv_d = nc.dram_tensor("wv", [HIDDEN, IN_PC], FP32, kind="ExternalInput")
    wg_d = nc.dram_tensor("wg", [HIDDEN, IN_PC], FP32, kind="ExternalInput")
    wo_d = nc.dram_tensor("wo", [IN_PC, HIDDEN], FP32, kind="ExternalInput")
    normw_d = nc.dram_tensor("normw", [HPC, P, 1], FP32, kind="ExternalInput")
    qdec_d = nc.dram_tensor("qdec", [HPC, P, BLOCK], FP32, kind="ExternalInput")
    dmask0_d = nc.dram_tensor("dmask0", [HPC, P, BLOCK], FP32, kind="ExternalInput")
    dmask1_d = nc.dram_tensor("dmask1", [HPC, P, BLOCK], FP32, kind="ExternalInput")
    kdec_d = nc.dram_tensor("kdec", [HPC, 2, P, 1], FP32, kind="ExternalInput")
    bd_d = nc.dram_tensor("bd", [P, HPC, 1], FP32, kind="ExternalInput")
    ident_d = nc.dram_tensor("ident", [P, P], FP32, kind="ExternalInput")
    ones_d = nc.dram_tensor("ones", [P, 1], FP32, kind="ExternalInput")
    kv0_d = nc.dram_tensor("kv0", [HPC, P, P], FP32, kind="ExternalInput")

    pout_d = nc.dram_tensor("pout", [SEQ, HIDDEN], FP32, kind="ExternalOutput")
    ssq_d = nc.dram_tensor("ssq", [1, SEQ], FP32, kind="ExternalOutput")

    # internal DRAM intermediates
    qT_d = nc.dram_tensor("qT_i", [IN_PC, SEQ], FP32R)
    kT_d = nc.dram_tensor("kT_i", [IN_PC, SEQ], FP32R)
    v_d = nc.dram_tensor("v_i", [SEQ, IN_PC], FP32R)
    gT_d = nc.dram_tensor("gT_i", [IN_PC, SEQ], FP32)

    with tile.TileContext(nc) as tc, ExitStack() as ctx:
        const = ctx.enter_context(tc.tile_pool(name="const", bufs=1))
        psum = ctx.enter_context(tc.tile_pool(name="psum", bufs=1, space="PSUM"))

        ident_t = const.tile([P, P], FP32R)
        nc.sync.dma_start(ident_t[:], ident_d[:].bitcast(FP32R))
        ones_t = const.tile([P, 1], FP32R)
        nc.sync.dma_start(ones_t[:], ones_d[:].bitcast(FP32R))
        qdec_t = const.tile([P, HPC, BLOCK], FP32)
        nc.sync.dma_start(qdec_t[:], qdec_d[:].rearrange("h p i -> p h i"))
        dmask0_t = const.tile([P, HPC, BLOCK], FP32)
        nc.sync.dma_start(dmask0_t[:], dmask0_d[:].rearrange("h p i -> p h i"))
        dmask1_t = const.tile([P, HPC, BLOCK], FP32)
        nc.sync.dma_start(dmask1_t[:], dmask1_d[:].rearrange("h p i -> p h i"))
        kdec_t = const.tile([P, HPC, 2, 1], FP32)
        nc.sync.dma_start(kdec_t[:], kdec_d[:].rearrange("h s p o -> p h s o"))
        normw_t = const.tile([P, HPC, 1], FP32)
        nc.sync.dma_start(normw_t[:], normw_d[:].rearrange("h p o -> p h o"))
        bd_t = const.tile([P, HPC, 1], FP32)
        nc.sync.dma_start(bd_t[:], bd_d[:])
        kv_t = const.tile([P, HPC, P], FP32R)

        for _rep in range(repeat):
            nc.sync.dma_start(kv_t[:], kv0_d[:].rearrange("h d e -> d h e").bitcast(FP32R))

            # ---------------- phase A: projections ----------------
            if "A" not in phases:
                pass
            else:
              with ExitStack() as actx:
                wpool = actx.enter_context(tc.tile_pool(name="wA", bufs=1))
                xpool = actx.enter_context(tc.tile_pool(name="xA", bufs=1))
                stA = actx.enter_context(tc.tile_pool(name="stA", bufs=1))

                wq_t = wpool.tile([P, HC, IN_PC], FP32R)
                nc.sync.dma_start(wq_t[:], wq_d[:].rearrange("(hc p) m -> p hc m", p=P).bitcast(FP32R))
                wk_t = wpool.tile([P, HC, IN_PC], FP32R)
                nc.sync.dma_start(wk_t[:], wk_d[:].rearrange("(hc p) m -> p hc m", p=P).bitcast(FP32R))
                wv_t = wpool.tile([P, HC, IN_PC], FP32R)
                nc.sync.dma_start(wv_t[:], wv_d[:].rearrange("(hc p) m -> p hc m", p=P).bitcast(FP32R))
                wg_t = wpool.tile([P, HC, IN_PC], FP32R)
                nc.sync.dma_start(wg_t[:], wg_d[:].rearrange("(hc p) m -> p hc m", p=P).bitcast(FP32R))

                xt_r = xt_d[:].rearrange("(hc p) n -> p hc n", p=P).bitcast(FP32R)
                HH = HC // 2

                for t in range(NT_A):
                    xt_h = []
                    for half in range(2):
                        xh = xpool.tile([P, HH, CH_A], FP32R, tag="xt", bufs=3)
                        nc.sync.dma_start(
                            xh[:], xt_r[:, half * HH:(half + 1) * HH,
                                        t * CH_A:(t + 1) * CH_A])
                        xt_h.append(xh)

                    for w_t, actf, dst, sdt, tg in (
                        (wq_t, AF.Silu, qT_d, FP32R, "sq"),
                        (wk_t, AF.Silu, kT_d, FP32R, "sk"),
                        (wg_t, AF.Sigmoid, gT_d, FP32, "sg"),
                    ):
                        for cc in range(4):
                            ps = psum.tile([P, IN_PC], FP32, tag="psA", bufs=2)
                            for hc in range(HC):
                                nc.tensor.matmul(
                                    ps[:],
                                    w_t[:, hc, cc * P:(cc + 1) * P],
                                    xt_h[hc // HH][:, hc % HH, :],
                                    start=(hc == 0), stop=(hc == HC - 1))
                            sb = stA.tile([P, CH_A], sdt, tag=tg, bufs=2)
                            nc.scalar.activation(sb[:], ps[:], actf)
                            nc.sync.dma_start(
                                dst[cc * P:(cc + 1) * P, t * CH_A:(t + 1) * CH_A], sb[:])

                    for t2 in range(4):
                        ps = psum.tile([P, IN_PC], FP32, tag="psA", bufs=2)
                        for hc in range(HC):
                            nc.tensor.matmul(
                                ps[:],
                                xt_h[hc // HH][:, hc % HH, t2 * P:(t2 + 1) * P],
                                wv_t[:, hc, :],
                                start=(hc == 0), stop=(hc == HC - 1))
                        sb = stA.tile([P, IN_PC], FP32R, tag="sv", bufs=2)
                        nc.scalar.activation(sb[:], ps[:], AF.Silu)
                        nc.sync.dma_start(
                            v_d[t * CH_A + t2 * P: t * CH_A + (t2 + 1) * P, :], sb[:])

            # ---------------- phase B: attention + gating + out ----------------
            if "B" not in phases:
                pass
            else:
              with ExitStack() as bctx:
                wBpool = bctx.enter_context(tc.tile_pool(name="wB", bufs=1))
                sB = bctx.enter_context(tc.tile_pool(name="sB", bufs=1))
                wk_b = bctx.enter_context(tc.tile_pool(name="wkB", bufs=1))

                wo_t = wBpool.tile([P, HPC, HIDDEN], FP32R)
                nc.sync.dma_start(wo_t[:], wo_d[:].rearrange("(h p) n -> p h n", p=P).bitcast(FP32R))

                qT_r = qT_d[:].rearrange("(h p) n -> p h n", p=P)
                kT_r = kT_d[:].rearrange("(h p) n -> p h n", p=P)
                gT_r = gT_d[:].rearrange("(h p) n -> p h n", p=P)

                for t in range(NT_B):
                    tsl = slice(t * CH_B, (t + 1) * CH_B)
                    q_t = sB.tile([P, HPC, CH_B], FP32R, tag="q", bufs=2)
                    nc.sync.dma_start(q_t[:], qT_r[:, :, tsl])
                    k_t = sB.tile([P, HPC, CH_B], FP32R, tag="k", bufs=2)
                    nc.sync.dma_start(k_t[:], kT_r[:, :, tsl])
                    g_t = sB.tile([P, HPC, CH_B], FP32, tag="g", bufs=2)
                    nc.sync.dma_start(g_t[:], gT_r[:, :, tsl])
                    v_t = sB.tile([P, 4, IN_PC], FP32R, tag="v", bufs=2)
                    nc.sync.dma_start(
                        v_t[:], v_d[tsl, :].rearrange("(s p) c -> p s c", p=P))

                    attn_t = wk_b.tile([P, HPC, CH_B], FP32, tag="attn", bufs=2)

                    for b in range(CH_B // BLOCK):
                        t0 = b * BLOCK
                        for h in range(HPC):
                            hsl = slice(h * P, (h + 1) * P)
                            # scores (transposed): sT[j, i] = k_j . q_i
                            ps0 = psum.tile([P, BLOCK], FP32, tag="ps_s", bufs=2)
                            nc.tensor.matmul(ps0[:], k_t[:, h, t0:t0 + P],
                                             q_t[:, h, t0:t0 + BLOCK],
                                             start=True, stop=True)
                            ps1 = psum.tile([P, BLOCK], FP32, tag="ps_s", bufs=2)
                            nc.tensor.matmul(ps1[:], k_t[:, h, t0 + P:t0 + BLOCK],
                                             q_t[:, h, t0:t0 + BLOCK],
                                             start=True, stop=True)
                            s0 = wk_b.tile([P, BLOCK], FP32R, tag="s0", bufs=2)
                            nc.vector.tensor_mul(s0[:], ps0[:], dmask0_t[:, h, :])
                            s1 = wk_b.tile([P, BLOCK], FP32R, tag="s1", bufs=2)
                            nc.vector.tensor_mul(s1[:], ps1[:], dmask1_t[:, h, :])
                            qd = wk_b.tile([P, BLOCK], FP32R, tag="qd", bufs=2)
                            nc.vector.tensor_mul(qd[:], q_t[:, h, t0:t0 + BLOCK].bitcast(FP32),
                                                 qdec_t[:, h, :])
                            # k natural (transposed back) with k-decay folded in
                            kn = []
                            for sub in range(2):
                                pst = psum.tile([P, P], FP32, tag="ps_tr", bufs=1)
                                nc.tensor.transpose(
                                    pst[:].bitcast(FP32R),
                                    k_t[:, h, t0 + sub * P:t0 + (sub + 1) * P],
                                    ident_t[:])
                                knt = wk_b.tile([P, P], FP32R, tag=f"kn{sub}", bufs=2)
                                nc.scalar.activation(knt[:], pst[:], AF.Copy,
                                                     scale=kdec_t[:, h, sub, :])
                                kn.append(knt)
                            # attention output (transposed): inter + intra
                            pso = psum.tile([P, BLOCK], FP32, tag="ps_o", bufs=2)
                            nc.tensor.matmul(pso[:], kv_t[:, h, :], qd[:],
                                             start=True, stop=False)
                            nc.tensor.matmul(pso[:], v_t[:, 2 * b, hsl], s0[:],
                                             start=False, stop=False)
                            nc.tensor.matmul(pso[:], v_t[:, 2 * b + 1, hsl], s1[:],
                                             start=False, stop=True)
                            nc.scalar.copy(attn_t[:, h, t0:t0 + BLOCK], pso[:])
                            # kv update: kv = bd*kv + (k*kdec)^T v
                            psk = psum.tile([P, P], FP32, tag="ps_kv", bufs=1)
                            nc.tensor.matmul(psk[:], kn[0][:], v_t[:, 2 * b, hsl],
                                             start=True, stop=False)
                            nc.tensor.matmul(psk[:], kn[1][:], v_t[:, 2 * b + 1, hsl],
                                             start=False, stop=True)
                            nc.vector.tensor_scalar_mul(
                                kv_t[:, h, :], kv_t[:, h, :].bitcast(FP32), bd_t[:, h, :])
                            nc.vector.tensor_add(
                                kv_t[:, h, :], kv_t[:, h, :].bitcast(FP32), psk[:])

                    # gating + norm weight + squares
                    gA_t = wk_b.tile([P, HPC, CH_B], FP32R, tag="gA", bufs=2)
                    sq_t = wk_b.tile([P, HPC, CH_B], FP32R, tag="sq", bufs=2)
                    for h in range(HPC):
                        nc.vector.tensor_mul(gA_t[:, h, :], attn_t[:, h, :], g_t[:, h, :])
                        nc.vector.tensor_scalar_mul(
                            gA_t[:, h, :], gA_t[:, h, :].bitcast(FP32), normw_t[:, h, :])
                        nc.vector.tensor_mul(sq_t[:, h, :], attn_t[:, h, :], attn_t[:, h, :])

                    # ssq = ones^T @ sq  (sum over this core's inner channels)
                    pss = psum.tile([1, CH_B], FP32, tag="psA", bufs=2)
                    for h in range(HPC):
                        nc.tensor.matmul(pss[:], ones_t[:], sq_t[:, h, :],
                                         start=(h == 0), stop=(h == HPC - 1))
                    ssb = wk_b.tile([1, CH_B], FP32, tag="ssb", bufs=2)
                    nc.scalar.copy(ssb[:], pss[:])
                    nc.sync.dma_start(ssq_d[:, tsl], ssb[:])

                    # out projection
                    for m in range(4):
                        ob = wk_b.tile([P, HIDDEN], FP32, tag="ob", bufs=2)
                        for nt in range(4):
                            pso2 = psum.tile([P, 512], FP32, tag="psA", bufs=2)
                            for h in range(HPC):
                                nc.tensor.matmul(
                                    pso2[:],
                                    gA_t[:, h, m * P:(m + 1) * P],
                                    wo_t[:, h, nt * 512:(nt + 1) * 512],
                                    start=(h == 0), stop=(h == HPC - 1))
                            nc.scalar.copy(ob[:, nt * 512:(nt + 1) * 512], pso2[:])
                        nc.sync.dma_start(
                            pout_d[t * CH_B + m * P: t * CH_B + (m + 1) * P, :], ob[:])

    nc.compile()
    return nc


_NC_CACHE = {}


def _get_nc(repeat=1, phases="AB"):
    key = (repeat, phases)
    if key not in _NC_CACHE:
        _NC_CACHE[key] = build_nc(repeat, phases)
    return _NC_CACHE[key]


def make_in_maps(inputs):
    hs = np.ascontiguousarray(np.asarray(inputs["hidden_states"], dtype=np.float32))
    w_qkv = np.asarray(inputs["w_qkv"], dtype=np.float32)
    w_gate = np.asarray(inputs["w_gate"], dtype=np.float32)
    w_out = np.asarray(inputs["w_out"], dtype=np.float32)
    norm_weight = np.asarray(inputs["norm_weight"], dtype=np.float32)
    slope_rate = np.asarray(inputs["slope_rate"], dtype=np.float32).reshape(NUM_HEADS)
    kv_cache = np.asarray(inputs["kv_cache"], dtype=np.float32)

    xt = np.ascontiguousarray(hs.T)                      # [HIDDEN, SEQ]
    wq3 = w_qkv.reshape(HIDDEN, NUM_HEADS, 3 * HEAD_DIM)
    ident = np.eye(P, dtype=np.float32)
    ones = np.ones((P, 1), dtype=np.float32)
    idx = np.arange(BLOCK, dtype=np.float64)

    in_maps = []
    for c in range(N_CORES):
        heads = range(c * HPC, (c + 1) * HPC)
        s = slope_rate[c * HPC:(c + 1) * HPC].astype(np.float64)  # [HPC]
        wq = np.ascontiguousarray(
            wq3[:, c * HPC:(c + 1) * HPC, 0:HEAD_DIM].reshape(HIDDEN, IN_PC))
        wk = np.ascontiguousarray(
            wq3[:, c * HPC:(c + 1) * HPC, HEAD_DIM:2 * HEAD_DIM].reshape(HIDDEN, IN_PC))
        wv = np.ascontiguousarray(
            wq3[:, c * HPC:(c + 1) * HPC, 2 * HEAD_DIM:3 * HEAD_DIM].reshape(HIDDEN, IN_PC))
        wg = np.ascontiguousarray(w_gate[:, c * IN_PC:(c + 1) * IN_PC])
        wo = np.ascontiguousarray(w_out[c * IN_PC:(c + 1) * IN_PC, :])
        normw = np.ascontiguousarray(
            norm_weight[c * IN_PC:(c + 1) * IN_PC].reshape(HPC, P, 1))

        diff = idx[:128, None] - idx[None, :]            # j - i restricted later
        # dmask0[h, j, i] = exp(-s (i - j)) for i >= j (j in 0..127, i in 0..255)
        jj = idx[:128][:, None]                          # [128,1]
        ii = idx[None, :]                                # [1,256]
        d0 = np.exp(-s[:, None, None] * (ii - jj)) * (ii >= jj)
        dmask0 = d0.astype(np.float32)                   # [HPC,128,256]
        # dmask1[h, j', i] for abs j = j'+128: zero for i<128, else dmask0[j', i-128]
        dmask1 = np.zeros((HPC, P, BLOCK), dtype=np.float32)
        dmask1[:, :, P:] = dmask0[:, :, :P]
        qdec = np.broadcast_to(
            np.exp(-s[:, None] * (idx[None, :] + 1.0))[:, None, :],
            (HPC, P, BLOCK)).astype(np.float32)
        kdec = np.exp(-s[:, None] * (BLOCK - 1.0 - idx[None, :]))  # [HPC, 256]
        kdec = kdec.reshape(HPC, 2, P, 1).astype(np.float32)
        bd = np.broadcast_to(
            np.exp(-s * BLOCK).astype(np.float32)[None, :, None], (P, HPC, 1))
        kv0 = np.ascontiguousarray(kv_cache[c * HPC:(c + 1) * HPC])

        in_maps.append({
            "xt": xt, "wq": wq, "wk": wk, "wv": wv, "wg": wg, "wo": wo,
            "normw": normw, "qdec": np.ascontiguousarray(qdec),
            "dmask0": dmask0, "dmask1": dmask1, "kdec": kdec,
            "bd": np.ascontiguousarray(bd), "ident": ident, "ones": ones,
            "kv0": kv0,
        })
    return in_maps


def combine_outputs(results):
    pout = np.zeros((SEQ, HIDDEN), dtype=np.float64)
    ssq = np.zeros((SEQ,), dtype=np.float64)
    for r in results:
        pout += r["pout"].astype(np.float64)
        ssq += r["ssq"].reshape(SEQ).astype(np.float64)
    var = ssq / INNER
    scale = 1.0 / np.sqrt(var + EPS)
    return (pout * scale[:, None]).astype(np.float32)


def kernel(**inputs):
    nc = _get_nc(1)
    in_maps = make_in_maps(inputs)
    res = run_bass_kernel_spmd(nc, in_maps, core_ids=list(range(N_CORES)))
    return combine_outputs(res.results)
